# revision 1
# baseline (speedup 1.0000x reference)
"""CrossAttentionLayer kernel for 8x Trainium2 NeuronCores.

Problem (hardcoded): B=2, S=4096, HIDDEN=4096, HEADS=32, HEAD_DIM=128,
SLOTS=128, LN eps 1e-5.  out = x + (softmax(LN(x)@Wq.T split-heads @ K.T
/ sqrt(128), masked) @ V merge-heads) @ Wout.T

Strategy: data-parallel over the 8192 (B*S) rows — 1024 rows per core,
core c takes batch c//4.  Everything on-device except layout prep:
  * transposed dataflow: x.T [k, s] tiles; LN stats (mean/var over k =
    partition axis) via ones-matmul partition-broadcast sums; xn = bf16
  * Q-proj: QT[n,s] = (Wq*gamma).T-stationary @ xn.T, bias (beta@Wq.T)
    added on psum->sbuf copy; fused per-head with attention
  * attention per head in [t,s]/[d,s] layout: scoresT = K_h.T-st @ QT_h;
    exp on ACT (mask as per-partition bias, 1/sqrt(128) as scale);
    denominator via ones-matmul; attnT = V_h-st @ expT, normalized by
    reciprocal on the psum->sbuf copy (bf16)
  * out-proj: outT[n,s] = Wout.T-tiles-stationary @ attnT accumulated
    over heads; residual added from x.T f32; output outT per core,
    transposed/concatenated on host.
All matmuls bf16 x bf16 -> fp32 psum, N=512.
"""
import numpy as np
import ml_dtypes
import concourse.bass as bass
import concourse.mybir as mybir
import concourse.tile as tile
from concourse.vector_clock import ScopedClock

F32 = mybir.dt.float32
BF16 = mybir.dt.bfloat16
AF = mybir.ActivationFunctionType

B, S, HID, HEADS, DH, SLOTS = 2, 4096, 4096, 32, 128, 128
NC_ = 8
SC = B * S // NC_          # rows per core = 1024
KT = HID // 128            # 32 k-tiles
NT = HID // 128            # 32 n-tiles (= heads for Q)
NSL = SC // 512            # 2 moving slices of 512
EPS = 1e-5
SCALE = DH ** -0.5

_ws_counter = [0]


def _split_waits(nc, maxw=1):
    """This walrus build rejects >1 sync-wait per instruction: hoist
    extras into same-engine no-ops placed just before the instruction."""
    n = 0
    for f in nc.m.functions:
        for blk in f.blocks:
            insts = list(blk.instructions)
            out, dirty = [], False
            for inst in insts:
                si = inst.sync_info
                waits = list(si.on_wait) if (si is not None and si.on_wait) else []
                if len(waits) > maxw:
                    ups = list(si.on_update or [])
                    for i in range(maxw, len(waits), maxw):
                        _ws_counter[0] += 1
                        nop = mybir.InstNoOp(
                            name=f"I-ws{_ws_counter[0]}", ins=[], outs=[]
                        )
                        nop.engine = inst.engine
                        nop.sync_info = mybir.SyncInfo(
                            on_wait=waits[i : i + maxw], on_update=[]
                        )
                        out.append(nop)
                        n += 1
                    inst.sync_info = mybir.SyncInfo(
                        on_wait=waits[:maxw], on_update=ups
                    )
                    dirty = True
                out.append(inst)
            if dirty:
                blk.instructions = out
    return n


def _patch_tile_drain():
    import concourse.tile as tile_mod

    def _patched(self, tick_clock, wait_clock):
        nc = self.nc
        drain_inst = nc.sync.drain()
        wait_clock.add_sem_waits(
            drain_inst.ins, ScopedClock({None: tick_clock.global_clock})
        )
        inst = drain_inst.ins
        si = inst.sync_info
        waits = list(si.on_wait or []) if si is not None else []
        if len(waits) > 1:
            ups = list(si.on_update or []) if si is not None else []
            inst.sync_info = mybir.SyncInfo(on_wait=waits[:1], on_update=ups)
            for i in range(1, len(waits)):
                nop = nc.sync.nop()
                nop.ins.sync_info = mybir.SyncInfo(
                    on_wait=waits[i : i + 1], on_update=[]
                )
        nc.all_engine_barrier()
        assert self.sems is not None
        popped = nc._tile_sem_poison_stack.pop()
        assert popped is self._sem_poison
        nc.clear_and_free_semaphores(list(self.sems.allocated().values()))
        nc.all_engine_barrier()

    tile_mod.TileContext._drain_and_barrier = _patched


def build_nc():
    _patch_tile_drain()
    nc = bass.Bass()

    xtb_in = nc.dram_tensor("xtb", [HID, SC], BF16, kind="ExternalInput")
    xtf_in = nc.dram_tensor("xtf", [HID, SC], F32, kind="ExternalInput")
    wqt_in = nc.dram_tensor("wqt", [HID, HID], BF16, kind="ExternalInput")
    bq_in = nc.dram_tensor("bq", [128, NT], F32, kind="ExternalInput")
    wot_in = nc.dram_tensor("wot", [HID, HID], BF16, kind="ExternalInput")
    kt_in = nc.dram_tensor("ktt", [HEADS, DH, SLOTS], BF16, kind="ExternalInput")
    v_in = nc.dram_tensor("vv", [HEADS, SLOTS, DH], BF16, kind="ExternalInput")
    mb_in = nc.dram_tensor("mb", [SLOTS, 1], F32, kind="ExternalInput")
    out_t = nc.dram_tensor("outt", [HID, SC], F32, kind="ExternalOutput")

    with tile.TileContext(nc) as tc:
        with tc.tile_pool(name="persist", bufs=1) as P:
            ones = P.tile([128, 128], BF16, tag="ones")
            nc.vector.memset(ones[:], 1.0)
            eps_t = P.tile([128, 1], F32, tag="eps")
            nc.vector.memset(eps_t[:], EPS)
            kt_all = P.tile([128, HEADS, SLOTS], BF16, tag="kt")
            nc.sync.dma_start(
                kt_all[:], kt_in[:].rearrange("h d t -> d h t")
            )
            v_all = P.tile([128, HEADS, DH], BF16, tag="v")
            nc.sync.dma_start(v_all[:], v_in[:].rearrange("h t d -> t h d"))
            mb = P.tile([128, 1], F32, tag="mb")
            nc.sync.dma_start(mb[:], mb_in[:])
            bq = P.tile([128, NT], F32, tag="bq")
            nc.sync.dma_start(bq[:], bq_in[:])

            rstd_b = P.tile([128, SC], F32, tag="rstd")
            mrs_b = P.tile([128, SC], F32, tag="mrs")
            xn = [P.tile([128, SC], BF16, tag=f"xn{k}", name=f"xn{k}") for k in range(KT)]
            att = [P.tile([128, SC], BF16, tag=f"att{h}", name=f"att{h}") for h in range(HEADS)]

            # ---------- phase 1: LN stats ----------
            with (
                tc.tile_pool(name="xs", bufs=6) as XS,
                tc.tile_pool(name="sqp", bufs=4) as SQ,
                tc.tile_pool(name="stps", bufs=1, space="PSUM") as STP,
                tc.tile_pool(name="stsb", bufs=2) as STS,
            ):
                sum_ps = [STP.tile([128, 512], F32, tag=f"sum{sl}", name=f"sum{sl}") for sl in range(NSL)]
                ssq_ps = [STP.tile([128, 512], F32, tag=f"ssq{sl}", name=f"ssq{sl}") for sl in range(NSL)]
                for k in range(KT):
                    xt = XS.tile([128, SC], BF16, tag="xt")
                    nc.sync.dma_start(xt[:], xtb_in[k * 128 : (k + 1) * 128, :])
                    sq = SQ.tile([128, SC], BF16, tag="sq")
                    nc.scalar.square(sq[:], xt[:])
                    for sl in range(NSL):
                        cs = slice(sl * 512, (sl + 1) * 512)
                        nc.tensor.matmul(
                            sum_ps[sl][:], ones[:], xt[:, cs],
                            start=(k == 0), stop=(k == KT - 1),
                        )
                        nc.tensor.matmul(
                            ssq_ps[sl][:], ones[:], sq[:, cs],
                            start=(k == 0), stop=(k == KT - 1),
                        )
                for sl in range(NSL):
                    cs = slice(sl * 512, (sl + 1) * 512)
                    mean = STS.tile([128, 512], F32, tag="mean")
                    nc.vector.tensor_scalar_mul(mean[:], sum_ps[sl][:], 1.0 / HID)
                    esq = STS.tile([128, 512], F32, tag="esq")
                    nc.vector.tensor_scalar_mul(esq[:], ssq_ps[sl][:], 1.0 / HID)
                    msq = STS.tile([128, 512], F32, tag="msq")
                    nc.vector.tensor_mul(msq[:], mean[:], mean[:])
                    var = STS.tile([128, 512], F32, tag="var")
                    nc.vector.tensor_sub(var[:], esq[:], msq[:])
                    std = STS.tile([128, 512], F32, tag="std")
                    nc.scalar.activation(std[:], var[:], AF.Sqrt, bias=eps_t[:])
                    nc.vector.reciprocal(rstd_b[:, cs], std[:])
                    nc.vector.tensor_mul(mrs_b[:, cs], mean[:], rstd_b[:, cs])

            # ---------- phase 2: xn ----------
            with (
                tc.tile_pool(name="xs2", bufs=6) as XS2,
                tc.tile_pool(name="tmpp", bufs=4) as TMP,
            ):
                for k in range(KT):
                    xt = XS2.tile([128, SC], BF16, tag="xt2")
                    nc.sync.dma_start(xt[:], xtb_in[k * 128 : (k + 1) * 128, :])
                    tmp = TMP.tile([128, SC], F32, tag="tmp")
                    nc.vector.tensor_mul(tmp[:], xt[:], rstd_b[:])
                    nc.vector.tensor_sub(xn[k][:], tmp[:], mrs_b[:])

            # ---------- phase 3: per-head Q-proj + attention ----------
            with (
                tc.tile_pool(name="wq", bufs=2) as WQ,
                tc.tile_pool(name="qps", bufs=1, space="PSUM") as QPS,
                tc.tile_pool(name="qsb", bufs=2) as QSB,
                tc.tile_pool(name="aps", bufs=1, space="PSUM") as APS,
                tc.tile_pool(name="expp", bufs=2) as EXP,
                tc.tile_pool(name="rcp", bufs=2) as RCP,
            ):
                for h in range(HEADS):
                    wq = WQ.tile([128, KT, 128], BF16, tag="wq")
                    nc.sync.dma_start(
                        wq[:],
                        wqt_in[:, h * 128 : (h + 1) * 128].rearrange(
                            "(kt p) n -> p kt n", p=128
                        ),
                    )
                    qt_ps = [QPS.tile([128, 512], F32, tag=f"qt{sl}", name=f"qtp{sl}") for sl in range(NSL)]
                    for k in range(KT):
                        for sl in range(NSL):
                            cs = slice(sl * 512, (sl + 1) * 512)
                            nc.tensor.matmul(
                                qt_ps[sl][:], wq[:, k, :], xn[k][:, cs],
                                start=(k == 0), stop=(k == KT - 1),
                            )
                    qt = QSB.tile([128, SC], BF16, tag="qt")
                    for sl in range(NSL):
                        cs = slice(sl * 512, (sl + 1) * 512)
                        nc.vector.tensor_scalar_add(
                            qt[:, cs], qt_ps[sl][:], bq[:, h : h + 1]
                        )
                    expt = EXP.tile([128, SC], BF16, tag="expt")
                    for sl in range(NSL):
                        cs = slice(sl * 512, (sl + 1) * 512)
                        sc_ps = APS.tile([128, 512], F32, tag=f"sc{sl}")
                        nc.tensor.matmul(
                            sc_ps[:], kt_all[:, h, :], qt[:, cs],
                            start=True, stop=True,
                        )
                        nc.scalar.activation(
                            expt[:, cs], sc_ps[:], AF.Exp,
                            bias=mb[:], scale=SCALE,
                        )
                    for sl in range(NSL):
                        cs = slice(sl * 512, (sl + 1) * 512)
                        den_ps = APS.tile([128, 512], F32, tag=f"den{sl}")
                        nc.tensor.matmul(
                            den_ps[:], ones[:], expt[:, cs], start=True, stop=True
                        )
                        rcp = RCP.tile([128, 512], F32, tag="rcp")
                        nc.vector.reciprocal(rcp[:], den_ps[:])
                        at_ps = APS.tile([128, 512], F32, tag=f"at{sl}")
                        nc.tensor.matmul(
                            at_ps[:], v_all[:, h, :], expt[:, cs],
                            start=True, stop=True,
                        )
                        nc.vector.tensor_mul(att[h][:, cs], at_ps[:], rcp[:])

            # ---------- phase 4: out-proj + residual ----------
            with (
                tc.tile_pool(name="wo", bufs=2) as WO,
                tc.tile_pool(name="ops", bufs=2, space="PSUM") as OPS,
                tc.tile_pool(name="xrs", bufs=2) as XRS,
                tc.tile_pool(name="osb", bufs=3) as OSB,
            ):
                for nt in range(NT):
                    wo = WO.tile([128, KT, 128], BF16, tag="wo")
                    nc.sync.dma_start(
                        wo[:],
                        wot_in[:, nt * 128 : (nt + 1) * 128].rearrange(
                            "(ht p) n -> p ht n", p=128
                        ),
                    )
                    xr = XRS.tile([128, SC], F32, tag="xr")
                    nc.sync.dma_start(xr[:], xtf_in[nt * 128 : (nt + 1) * 128, :])
                    o_ps = [OPS.tile([128, 512], F32, tag=f"o{sl}", name=f"op{sl}") for sl in range(NSL)]
                    for h in range(HEADS):
                        for sl in range(NSL):
                            cs = slice(sl * 512, (sl + 1) * 512)
                            nc.tensor.matmul(
                                o_ps[sl][:], wo[:, h, :], att[h][:, cs],
                                start=(h == 0), stop=(h == HEADS - 1),
                            )
                    for sl in range(NSL):
                        cs = slice(sl * 512, (sl + 1) * 512)
                        osb = OSB.tile([128, 512], F32, tag="osb")
                        nc.vector.tensor_add(osb[:], o_ps[sl][:], xr[:, cs])
                        nc.sync.dma_start(
                            out_t[nt * 128 : (nt + 1) * 128, cs], osb[:]
                        )

    _split_waits(nc)
    return nc


_NC_CACHE = None
_LAST_IN_MAPS = None


def kernel(
    hidden_states, memory_keys, memory_values, attention_mask, Wq, Wout,
    ln_gamma, ln_beta,
):
    global _NC_CACHE
    if _NC_CACHE is None:
        _NC_CACHE = build_nc()
    nc = _NC_CACHE

    f32 = np.float32
    bf16 = ml_dtypes.bfloat16
    x = np.asarray(hidden_states, dtype=f32).reshape(B * S, HID)
    gamma = np.asarray(ln_gamma, dtype=f32)
    beta = np.asarray(ln_beta, dtype=f32)
    Wq = np.asarray(Wq, dtype=f32)
    Wout = np.asarray(Wout, dtype=f32)

    wqt = np.ascontiguousarray((Wq * gamma[None, :]).T).astype(bf16)
    bq = np.ascontiguousarray((Wq @ beta).reshape(NT, 128).T).astype(f32)
    wot = np.ascontiguousarray(Wout.T).astype(bf16)

    kts, vs, mbs = [], [], []
    for b in range(B):
        kb = np.asarray(memory_keys[b], dtype=f32).reshape(SLOTS, HEADS, DH)
        vb = np.asarray(memory_values[b], dtype=f32).reshape(SLOTS, HEADS, DH)
        kts.append(np.ascontiguousarray(kb.transpose(1, 2, 0)).astype(bf16))
        vs.append(np.ascontiguousarray(vb.transpose(1, 0, 2)).astype(bf16))
        m = np.asarray(attention_mask[b]).astype(bool)
        mbs.append(np.where(m, 0.0, -1e30).astype(f32).reshape(SLOTS, 1))

    in_maps = []
    for c in range(NC_):
        rows = slice(c * SC, (c + 1) * SC)
        xt = np.ascontiguousarray(x[rows].T)  # [HID, SC] f32
        b = (c * SC) // S
        in_maps.append(
            dict(
                xtb=xt.astype(bf16),
                xtf=xt,
                wqt=wqt,
                bq=bq,
                wot=wot,
                ktt=kts[b],
                vv=vs[b],
                mb=mbs[b],
            )
        )

    global _LAST_IN_MAPS
    _LAST_IN_MAPS = in_maps
    from concourse import bass2jax

    results = bass2jax.run_bass_via_pjrt(nc, in_maps, n_cores=NC_)

    out = np.empty((B * S, HID), dtype=f32)
    for c in range(NC_):
        out[c * SC : (c + 1) * SC] = results[c]["outt"].T
    return out.reshape(B, S, HID)



# revision 2
# speedup vs baseline: 1.6072x; 1.6072x over previous
"""CrossAttentionLayer kernel for 8x Trainium2 NeuronCores, fp8 edition.

Problem (hardcoded): B=2, S=4096, HIDDEN=4096, HEADS=32, HEAD_DIM=128,
SLOTS=128, LN eps 1e-5.  out = x + (softmax(LN(x)@Wq.T split-heads @ K.T
/ sqrt(128), masked) @ V merge-heads) @ Wout.T

Strategy: data-parallel over the 8192 (B*S) rows — 1024 rows per core,
core c takes batch c//4.  Since SLOTS == HEAD_DIM and HEADS*SLOTS ==
HIDDEN, both halves of the layer collapse into single 4096x4096 GEMMs
with host-precomputed merged weights:

  scores[(h,t), s] = sum_i A[i,(h,t)] * xhat[i, s]      A = (Wq*gamma).T @ K.T
  out[j, s]        = sum_(h,t) VWo[(h,t), j] * P[(h,t), s]  VWo = V @ Wout.T

P = softmax-normalized probs scaled by 128 (keeps e4m3 in normal range;
the 1/128 is folded into the output evacuation).  LN mean-subtraction
rides an augmented contraction row (-mu*r at partition 0 of an extra
k-plane pair, colA in A); beta and the attention mask ride the exp bias
(per-partition [t] bias per head).  Residual comes from raw-x bf16 tiles
kept in SBUF.

Both big GEMMs run fp8e4 with MatmulPerfMode.DoubleRow ([128, 2, *]
plane-pair operands, 256-deep contraction per instruction) for 2x+
tensor-engine throughput over bf16.
"""
import numpy as np
import ml_dtypes
import concourse.bass as bass
import concourse.mybir as mybir
import concourse.tile as tile
from concourse.vector_clock import ScopedClock

F32 = mybir.dt.float32
BF16 = mybir.dt.bfloat16
FP8 = mybir.dt.float8e4
AF = mybir.ActivationFunctionType
ALU = mybir.AluOpType
DR = mybir.MatmulPerfMode.DoubleRow
E4 = ml_dtypes.float8_e4m3

B, S, HID, HEADS, DH, SLOTS = 2, 4096, 4096, 32, 128, 128
NC_ = 8
SC = B * S // NC_          # rows per core = 1024
KT = HID // 128            # 32 k-tiles
KP = KT // 2 + 1           # 17 DoubleRow pairs (incl. augmented pair)
NT = HID // 128            # 32 out-blocks
HP = HEADS // 2            # 16 head pairs
NSL = SC // 512            # 2 moving slices of 512
EPS = 1e-5
SCALE = DH ** -0.5
PSC = 128.0                # prob scale: P = PSC * softmax

_ws_counter = [0]


def _split_waits(nc, maxw=1):
    """This walrus build rejects >1 sync-wait per instruction: hoist
    extras into same-engine no-ops placed just before the instruction."""
    n = 0
    for f in nc.m.functions:
        for blk in f.blocks:
            insts = list(blk.instructions)
            out, dirty = [], False
            for inst in insts:
                si = inst.sync_info
                waits = list(si.on_wait) if (si is not None and si.on_wait) else []
                if len(waits) > maxw:
                    ups = list(si.on_update or [])
                    for i in range(maxw, len(waits), maxw):
                        _ws_counter[0] += 1
                        nop = mybir.InstNoOp(
                            name=f"I-ws{_ws_counter[0]}", ins=[], outs=[]
                        )
                        nop.engine = inst.engine
                        nop.sync_info = mybir.SyncInfo(
                            on_wait=waits[i : i + maxw], on_update=[]
                        )
                        out.append(nop)
                        n += 1
                    inst.sync_info = mybir.SyncInfo(
                        on_wait=waits[:maxw], on_update=ups
                    )
                    dirty = True
                out.append(inst)
            if dirty:
                blk.instructions = out
    return n


def _patch_tile_drain():
    import concourse.tile as tile_mod

    def _patched(self, tick_clock, wait_clock):
        nc = self.nc
        drain_inst = nc.sync.drain()
        wait_clock.add_sem_waits(
            drain_inst.ins, ScopedClock({None: tick_clock.global_clock})
        )
        inst = drain_inst.ins
        si = inst.sync_info
        waits = list(si.on_wait or []) if si is not None else []
        if len(waits) > 1:
            ups = list(si.on_update or []) if si is not None else []
            inst.sync_info = mybir.SyncInfo(on_wait=waits[:1], on_update=ups)
            for i in range(1, len(waits)):
                nop = nc.sync.nop()
                nop.ins.sync_info = mybir.SyncInfo(
                    on_wait=waits[i : i + 1], on_update=[]
                )
        nc.all_engine_barrier()
        assert self.sems is not None
        popped = nc._tile_sem_poison_stack.pop()
        assert popped is self._sem_poison
        nc.clear_and_free_semaphores(list(self.sems.allocated().values()))
        nc.all_engine_barrier()

    tile_mod.TileContext._drain_and_barrier = _patched


def build_nc():
    _patch_tile_drain()
    nc = bass.Bass()

    xtb_in = nc.dram_tensor("xtb", [HID, SC], BF16, kind="ExternalInput")
    aq_in = nc.dram_tensor("aq", [HEADS, 128, 2 * KP, 128], FP8, kind="ExternalInput")
    vwq_in = nc.dram_tensor("vwq", [NT, 128, HEADS, 128], FP8, kind="ExternalInput")
    bias_in = nc.dram_tensor("biasv", [128, HEADS], F32, kind="ExternalInput")
    out_t = nc.dram_tensor("outt", [HID, SC], F32, kind="ExternalOutput")

    with tile.TileContext(nc) as tc:
        with tc.tile_pool(name="persist", bufs=1) as P:
            ones = P.tile([128, 128], BF16, tag="ones")
            nc.vector.memset(ones[:], 1.0)
            cden = P.tile([128, 128], BF16, tag="cden")
            nc.vector.memset(cden[:], 1.0 / PSC)
            eps_t = P.tile([128, 1], F32, tag="eps")
            nc.vector.memset(eps_t[:], EPS)
            biasv = P.tile([128, HEADS], F32, tag="biasv")
            nc.sync.dma_start(biasv[:], bias_in[:])

            rb = P.tile([128, SC], BF16, tag="rb")  # rstd broadcast
            xb = [P.tile([128, SC], BF16, tag=f"xb{k}", name=f"xb{k}")
                  for k in range(KT)]
            xq = [P.tile([128, 2, SC], FP8, tag=f"xq{kk}", name=f"xq{kk}")
                  for kk in range(KP)]
            pt = [P.tile([128, 2, SC], FP8, tag=f"pt{m}", name=f"pt{m}")
                  for m in range(HP)]
            nc.vector.memset(xq[KP - 1][:], 0.0)

            # ---------- phase 1: load x, LN stats ----------
            with (
                tc.tile_pool(name="sqp", bufs=4) as SQ,
                tc.tile_pool(name="stps", bufs=1, space="PSUM") as STP,
                tc.tile_pool(name="stsb", bufs=2) as STS,
            ):
                for k in range(KT):
                    nc.sync.dma_start(xb[k][:], xtb_in[k * 128 : (k + 1) * 128, :])
                sum_ps = [STP.tile([128, 512], F32, tag=f"sum{sl}", name=f"sum{sl}")
                          for sl in range(NSL)]
                ssq_ps = [STP.tile([128, 512], F32, tag=f"ssq{sl}", name=f"ssq{sl}")
                          for sl in range(NSL)]
                for k in range(KT):
                    sq = SQ.tile([128, SC], BF16, tag="sq")
                    nc.scalar.square(sq[:], xb[k][:])
                    for sl in range(NSL):
                        cs = slice(sl * 512, (sl + 1) * 512)
                        nc.tensor.matmul(
                            sum_ps[sl][:], ones[:], xb[k][:, cs],
                            start=(k == 0), stop=(k == KT - 1),
                        )
                        nc.tensor.matmul(
                            ssq_ps[sl][:], ones[:], sq[:, cs],
                            start=(k == 0), stop=(k == KT - 1),
                        )
                for sl in range(NSL):
                    cs = slice(sl * 512, (sl + 1) * 512)
                    mean = STS.tile([128, 512], F32, tag="mean")
                    nc.vector.tensor_scalar_mul(mean[:], sum_ps[sl][:], 1.0 / HID)
                    esq = STS.tile([128, 512], F32, tag="esq")
                    nc.vector.tensor_scalar_mul(esq[:], ssq_ps[sl][:], 1.0 / HID)
                    msq = STS.tile([128, 512], F32, tag="msq")
                    nc.vector.tensor_mul(msq[:], mean[:], mean[:])
                    var = STS.tile([128, 512], F32, tag="var")
                    nc.vector.tensor_sub(var[:], esq[:], msq[:])
                    std = STS.tile([128, 512], F32, tag="std")
                    nc.scalar.activation(std[:], var[:], AF.Sqrt, bias=eps_t[:])
                    with nc.allow_low_precision(reason="rstd bf16, fp8 gemm"):
                        nc.vector.reciprocal(rb[:, cs], std[:])
                    # augmented row: -mu*r at partition 0 of plane 32
                    with nc.allow_low_precision(reason="fp8 aug row"):
                        nc.vector.scalar_tensor_tensor(
                            xq[KP - 1][0:1, 0, cs], mean[0:1, :], -1.0,
                            rb[0:1, cs], ALU.mult, ALU.mult,
                        )

            # ---------- phase 2: quantize xhat to fp8 pairs ----------
            for k in range(KT):
                with nc.allow_low_precision(reason="fp8 gemm operand"):
                    nc.vector.tensor_mul(xq[k // 2][:, k % 2, :], xb[k][:], rb[:])

            # ---------- phase 3: scores GEMM + softmax ----------
            with (
                tc.tile_pool(name="wq", bufs=3) as WQ,
                tc.tile_pool(name="gps", bufs=2, space="PSUM") as GPS,
                tc.tile_pool(name="dps", bufs=1, space="PSUM") as DPS,
                tc.tile_pool(name="expp", bufs=3) as EXPP,
                tc.tile_pool(name="rcpp", bufs=4) as RCP,
            ):
                for h in range(HEADS):
                    at = WQ.tile([128, 2 * KP, 128], FP8, tag="at")
                    nc.sync.dma_start(at[:], aq_in[h])
                    expt = EXPP.tile([128, SC], BF16, tag="expt")
                    for sl in range(NSL):
                        cs = slice(sl * 512, (sl + 1) * 512)
                        g_ps = GPS.tile([128, 512], F32, tag=f"g{sl}")
                        for kk in range(KP):
                            nc.tensor.matmul(
                                g_ps[:], at[:, 2 * kk : 2 * kk + 2, :],
                                xq[kk][:, :, cs],
                                start=(kk == 0), stop=(kk == KP - 1),
                                perf_mode=DR,
                            )
                        nc.scalar.activation(
                            expt[:, cs], g_ps[:], AF.Exp,
                            bias=biasv[:, h : h + 1], scale=SCALE,
                        )
                        den_ps = DPS.tile([128, 512], F32, tag=f"d{sl}")
                        nc.tensor.matmul(
                            den_ps[:], cden[:], expt[:, cs], start=True, stop=True
                        )
                        rcp = RCP.tile([128, 512], BF16, tag="rcp")
                        with nc.allow_low_precision(reason="softmax denom"):
                            nc.vector.reciprocal(rcp[:], den_ps[:])
                        with nc.allow_low_precision(reason="fp8 probs"):
                            nc.vector.tensor_mul(
                                pt[h // 2][:, h % 2, cs], expt[:, cs], rcp[:]
                            )

            # ---------- phase 4: out GEMM + residual ----------
            with (
                tc.tile_pool(name="wo", bufs=3) as WO,
                tc.tile_pool(name="ops", bufs=2, space="PSUM") as OPS,
                tc.tile_pool(name="osb", bufs=4) as OSB,
            ):
                for nt in range(NT):
                    vt = WO.tile([128, HEADS, 128], FP8, tag="vt")
                    nc.sync.dma_start(vt[:], vwq_in[nt])
                    for sl in range(NSL):
                        cs = slice(sl * 512, (sl + 1) * 512)
                        o_ps = OPS.tile([128, 512], F32, tag=f"o{sl}")
                        for m in range(HP):
                            nc.tensor.matmul(
                                o_ps[:], vt[:, 2 * m : 2 * m + 2, :],
                                pt[m][:, :, cs],
                                start=(m == 0), stop=(m == HP - 1),
                                perf_mode=DR,
                            )
                        osb = OSB.tile([128, 512], F32, tag="osb")
                        nc.vector.scalar_tensor_tensor(
                            osb[:], o_ps[:], 1.0 / PSC, xb[nt][:, cs],
                            ALU.mult, ALU.add,
                        )
                        nc.sync.dma_start(
                            out_t[nt * 128 : (nt + 1) * 128, cs], osb[:]
                        )

    _split_waits(nc)
    return nc


_NC_CACHE = None
_LAST_IN_MAPS = None


def kernel(
    hidden_states, memory_keys, memory_values, attention_mask, Wq, Wout,
    ln_gamma, ln_beta,
):
    global _NC_CACHE
    if _NC_CACHE is None:
        _NC_CACHE = build_nc()
    nc = _NC_CACHE

    f32 = np.float32
    bf16 = ml_dtypes.bfloat16
    x = np.asarray(hidden_states, dtype=f32).reshape(B * S, HID)
    gamma = np.asarray(ln_gamma, dtype=f32)
    beta = np.asarray(ln_beta, dtype=f32)
    Wq = np.asarray(Wq, dtype=f32)
    Wout = np.asarray(Wout, dtype=f32)

    Wqg = Wq * gamma[None, :]                      # [o, i]
    W2h = Wqg.reshape(HEADS, DH, HID)              # [h, d, i]
    bq = (Wq @ beta).reshape(HEADS, DH)            # [h, d]
    Woh = Wout.reshape(HID, HEADS, DH).transpose(1, 2, 0)  # [h, d, j]

    aqs, vwqs, biases = [], [], []
    for b in range(B):
        Kb = np.asarray(memory_keys[b], dtype=f32)
        Vb = np.asarray(memory_values[b], dtype=f32)
        Kh = Kb.reshape(SLOTS, HEADS, DH).transpose(1, 0, 2)  # [h, t, d]
        Vh = Vb.reshape(SLOTS, HEADS, DH).transpose(1, 0, 2)  # [h, t, d]

        A = np.matmul(Kh, W2h).transpose(0, 2, 1)  # [h, i, t]
        colA = A.sum(axis=1)                       # [h, t]
        aq = np.zeros((HEADS, 128, 2 * KP, 128), dtype=f32)
        aq[:, :, :KT, :] = A.reshape(HEADS, KT, 128, 128).transpose(0, 2, 1, 3)
        aq[:, 0, KT, :] = colA
        aqs.append(aq.astype(E4))

        VWo = np.matmul(Vh, Woh)                   # [h, t, j]
        vwq = VWo.reshape(HEADS, SLOTS, NT, 128).transpose(2, 1, 0, 3)
        vwqs.append(np.ascontiguousarray(vwq).astype(E4))

        m = np.asarray(attention_mask[b]).astype(bool)           # [t]
        bqK = np.einsum("htd,hd->ht", Kh, bq)                    # [h, t]
        bias = SCALE * bqK.T + np.where(m, 0.0, -1e30)[:, None]  # [t, h]
        biases.append(np.ascontiguousarray(bias).astype(f32))

    in_maps = []
    for c in range(NC_):
        rows = slice(c * SC, (c + 1) * SC)
        xt = np.ascontiguousarray(x[rows].T).astype(bf16)  # [HID, SC]
        b = (c * SC) // S
        in_maps.append(
            dict(xtb=xt, aq=aqs[b], vwq=vwqs[b], biasv=biases[b])
        )

    global _LAST_IN_MAPS
    _LAST_IN_MAPS = in_maps
    from concourse import bass2jax

    results = bass2jax.run_bass_via_pjrt(nc, in_maps, n_cores=NC_)

    out = np.empty((B * S, HID), dtype=f32)
    for c in range(NC_):
        out[c * SC : (c + 1) * SC] = results[c]["outt"].T
    return out.reshape(B, S, HID)


# revision 9
# speedup vs baseline: 1.8316x; 1.1396x over previous
"""CrossAttentionLayer kernel for 8x Trainium2 NeuronCores, fp8 edition.

Problem (hardcoded): B=2, S=4096, HIDDEN=4096, HEADS=32, HEAD_DIM=128,
SLOTS=128, LN eps 1e-5.  out = x + (softmax(LN(x)@Wq.T split-heads @ K.T
/ sqrt(128), masked) @ V merge-heads) @ Wout.T

Strategy: data-parallel over the 8192 (B*S) rows — 1024 rows per core,
core c takes batch c//4.  Since SLOTS == HEAD_DIM and HEADS*SLOTS ==
HIDDEN, both halves of the layer collapse into single 4096x4096 GEMMs
with host-precomputed merged weights:

  scores[(h,t), s] = sum_i A[i,(h,t)] * xhat[i, s]      A = (Wq*gamma).T @ K.T
  out[j, s]        = sum_(h,t) VWo[(h,t), j] * P[(h,t), s]  VWo = V @ Wout.T

P = softmax-normalized probs scaled by 128 (keeps e4m3 in normal range;
the 1/128 is folded into the output evacuation).  LN mean-subtraction
rides an augmented contraction row (-mu*r at partition 0 of an extra
k-plane pair, colA in A); beta and the attention mask ride the exp bias
(per-partition [t] bias per head).  Residual comes from raw-x bf16 tiles
kept in SBUF.

Both big GEMMs run fp8e4 with MatmulPerfMode.DoubleRow ([128, 2, *]
plane-pair operands, 256-deep contraction per instruction) for 2x+
tensor-engine throughput over bf16.
"""
import numpy as np
import ml_dtypes
import concourse.bass as bass
import concourse.mybir as mybir
import concourse.tile as tile
from concourse.vector_clock import ScopedClock

F32 = mybir.dt.float32
BF16 = mybir.dt.bfloat16
FP8 = mybir.dt.float8e4
AF = mybir.ActivationFunctionType
ALU = mybir.AluOpType
DR = mybir.MatmulPerfMode.DoubleRow
E4 = ml_dtypes.float8_e4m3

B, S, HID, HEADS, DH, SLOTS = 2, 4096, 4096, 32, 128, 128
NC_ = 8
SC = B * S // NC_          # rows per core = 1024
KT = HID // 128            # 32 k-tiles
KP = KT // 2 + 1           # 17 DoubleRow pairs (incl. augmented pair)
NT = HID // 128            # 32 out-blocks
HP = HEADS // 2            # 16 head pairs
NSL = SC // 512            # 2 moving slices of 512
EPS = 1e-5
SCALE = DH ** -0.5
PSC = 128.0                # prob scale: P = PSC * softmax

_ws_counter = [0]


def _split_waits(nc, maxw=1):
    """This walrus build rejects >1 sync-wait per instruction: hoist
    extras into same-engine no-ops placed just before the instruction."""
    n = 0
    for f in nc.m.functions:
        for blk in f.blocks:
            insts = list(blk.instructions)
            out, dirty = [], False
            for inst in insts:
                si = inst.sync_info
                waits = list(si.on_wait) if (si is not None and si.on_wait) else []
                if len(waits) > maxw:
                    ups = list(si.on_update or [])
                    for i in range(maxw, len(waits), maxw):
                        _ws_counter[0] += 1
                        nop = mybir.InstNoOp(
                            name=f"I-ws{_ws_counter[0]}", ins=[], outs=[]
                        )
                        nop.engine = inst.engine
                        nop.sync_info = mybir.SyncInfo(
                            on_wait=waits[i : i + maxw], on_update=[]
                        )
                        out.append(nop)
                        n += 1
                    inst.sync_info = mybir.SyncInfo(
                        on_wait=waits[:maxw], on_update=ups
                    )
                    dirty = True
                out.append(inst)
            if dirty:
                blk.instructions = out
    return n


def _patch_tile_drain():
    import concourse.tile as tile_mod

    def _patched(self, tick_clock, wait_clock):
        nc = self.nc
        drain_inst = nc.sync.drain()
        wait_clock.add_sem_waits(
            drain_inst.ins, ScopedClock({None: tick_clock.global_clock})
        )
        inst = drain_inst.ins
        si = inst.sync_info
        waits = list(si.on_wait or []) if si is not None else []
        if len(waits) > 1:
            ups = list(si.on_update or []) if si is not None else []
            inst.sync_info = mybir.SyncInfo(on_wait=waits[:1], on_update=ups)
            for i in range(1, len(waits)):
                nop = nc.sync.nop()
                nop.ins.sync_info = mybir.SyncInfo(
                    on_wait=waits[i : i + 1], on_update=[]
                )
        nc.all_engine_barrier()
        assert self.sems is not None
        popped = nc._tile_sem_poison_stack.pop()
        assert popped is self._sem_poison
        nc.clear_and_free_semaphores(list(self.sems.allocated().values()))
        nc.all_engine_barrier()

    tile_mod.TileContext._drain_and_barrier = _patched


def build_nc():
    _patch_tile_drain()
    nc = bass.Bass()

    xtb_in = nc.dram_tensor("xtb", [HID, SC], BF16, kind="ExternalInput")
    aq_in = nc.dram_tensor("aq", [HEADS, 128, 2 * KP, 128], FP8, kind="ExternalInput")
    vwq_in = nc.dram_tensor("vwq", [NT, 128, HEADS, 128], FP8, kind="ExternalInput")
    bias_in = nc.dram_tensor("biasv", [128, HEADS], F32, kind="ExternalInput")
    out_t = nc.dram_tensor("outt", [HID, SC], F32, kind="ExternalOutput")

    with tile.TileContext(nc) as tc:
        with tc.tile_pool(name="persist", bufs=1) as P:
            ones = P.tile([128, 128], BF16, tag="ones")
            nc.vector.memset(ones[:], 1.0)
            cden = P.tile([128, 128], BF16, tag="cden")
            nc.vector.memset(cden[:], 1.0 / PSC)
            eps_t = P.tile([128, 1], F32, tag="eps")
            nc.vector.memset(eps_t[:], EPS)
            biasv = P.tile([128, HEADS], F32, tag="biasv")
            nc.sync.dma_start(biasv[:], bias_in[:])

            rb = P.tile([128, SC], BF16, tag="rb")  # rstd broadcast
            xb = [P.tile([128, SC], BF16, tag=f"xb{k}", name=f"xb{k}")
                  for k in range(KT)]
            xq = [P.tile([128, 2, SC], FP8, tag=f"xq{kk}", name=f"xq{kk}")
                  for kk in range(KP)]
            pt = [P.tile([128, 2, SC], FP8, tag=f"pt{m}", name=f"pt{m}")
                  for m in range(HP)]
            nc.vector.memset(xq[KP - 1][:], 0.0)

            # ---------- phase 1: load x, LN stats ----------
            with (
                tc.tile_pool(name="sqp", bufs=4) as SQ,
                tc.tile_pool(name="stps", bufs=1, space="PSUM") as STP,
                tc.tile_pool(name="stsb", bufs=2) as STS,
            ):
                for k in range(KT):
                    nc.sync.dma_start(xb[k][:], xtb_in[k * 128 : (k + 1) * 128, :])
                sum_ps = [STP.tile([128, 512], F32, tag=f"sum{sl}", name=f"sum{sl}")
                          for sl in range(NSL)]
                ssq_ps = [STP.tile([128, 512], F32, tag=f"ssq{sl}", name=f"ssq{sl}")
                          for sl in range(NSL)]
                for k in range(KT):
                    sq = SQ.tile([128, SC], BF16, tag="sq")
                    nc.scalar.square(sq[:], xb[k][:])
                    for sl in range(NSL):
                        cs = slice(sl * 512, (sl + 1) * 512)
                        nc.tensor.matmul(
                            sum_ps[sl][:], ones[:], xb[k][:, cs],
                            start=(k == 0), stop=(k == KT - 1),
                        )
                        nc.tensor.matmul(
                            ssq_ps[sl][:], ones[:], sq[:, cs],
                            start=(k == 0), stop=(k == KT - 1),
                        )
                for sl in range(NSL):
                    cs = slice(sl * 512, (sl + 1) * 512)
                    mean = STS.tile([128, 512], F32, tag="mean")
                    nc.vector.tensor_scalar_mul(mean[:], sum_ps[sl][:], 1.0 / HID)
                    esq = STS.tile([128, 512], F32, tag="esq")
                    nc.vector.tensor_scalar_mul(esq[:], ssq_ps[sl][:], 1.0 / HID)
                    msq = STS.tile([128, 512], F32, tag="msq")
                    nc.vector.tensor_mul(msq[:], mean[:], mean[:])
                    var = STS.tile([128, 512], F32, tag="var")
                    nc.vector.tensor_sub(var[:], esq[:], msq[:])
                    std = STS.tile([128, 512], F32, tag="std")
                    nc.scalar.activation(std[:], var[:], AF.Sqrt, bias=eps_t[:])
                    with nc.allow_low_precision(reason="rstd bf16, fp8 gemm"):
                        nc.vector.reciprocal(rb[:, cs], std[:])
                    # augmented row: -mu*r at partition 0 of plane 32
                    with nc.allow_low_precision(reason="fp8 aug row"):
                        nc.vector.scalar_tensor_tensor(
                            xq[KP - 1][0:1, 0, cs], mean[0:1, :], -1.0,
                            rb[0:1, cs], ALU.mult, ALU.mult,
                        )

            # ---------- phase 2: quantize xhat to fp8 pairs ----------
            for k in range(KT):
                with nc.allow_low_precision(reason="fp8 gemm operand"):
                    nc.vector.tensor_mul(xq[k // 2][:, k % 2, :], xb[k][:], rb[:])

            # ---------- phase 3: scores GEMM + softmax ----------
            with (
                tc.tile_pool(name="wq", bufs=3) as WQ,
                tc.tile_pool(name="gps", bufs=2, space="PSUM") as GPS,
                tc.tile_pool(name="dps", bufs=1, space="PSUM") as DPS,
                tc.tile_pool(name="expp", bufs=3) as EXPP,
                tc.tile_pool(name="rcpp", bufs=4) as RCP,
            ):
                for h in range(HEADS):
                    at = WQ.tile([128, 2 * KP, 128], FP8, tag="at")
                    nc.sync.dma_start(at[:], aq_in[h])
                    expt = EXPP.tile([128, SC], BF16, tag="expt")
                    for sl in range(NSL):
                        cs = slice(sl * 512, (sl + 1) * 512)
                        g_ps = GPS.tile([128, 512], F32, tag=f"g{sl}")
                        for kk in range(KP):
                            nc.tensor.matmul(
                                g_ps[:], at[:, 2 * kk : 2 * kk + 2, :],
                                xq[kk][:, :, cs],
                                start=(kk == 0), stop=(kk == KP - 1),
                                perf_mode=DR,
                            )
                        nc.scalar.activation(
                            expt[:, cs], g_ps[:], AF.Exp,
                            bias=biasv[:, h : h + 1], scale=SCALE,
                        )
                        den_ps = DPS.tile([128, 512], F32, tag=f"d{sl}")
                        nc.tensor.matmul(
                            den_ps[:], cden[:], expt[:, cs], start=True, stop=True
                        )
                        rcp = RCP.tile([128, 512], BF16, tag="rcp")
                        with nc.allow_low_precision(reason="softmax denom"):
                            nc.vector.reciprocal(rcp[:], den_ps[:])
                        with nc.allow_low_precision(reason="fp8 probs"):
                            nc.vector.tensor_mul(
                                pt[h // 2][:, h % 2, cs], expt[:, cs], rcp[:]
                            )

            # ---------- phase 4: out GEMM + residual ----------
            with (
                tc.tile_pool(name="wo", bufs=3) as WO,
                tc.tile_pool(name="ops", bufs=2, space="PSUM") as OPS,
                tc.tile_pool(name="osb", bufs=4) as OSB,
            ):
                for nt in range(NT):
                    vt = WO.tile([128, HEADS, 128], FP8, tag="vt")
                    nc.sync.dma_start(vt[:], vwq_in[nt])
                    for sl in range(NSL):
                        cs = slice(sl * 512, (sl + 1) * 512)
                        o_ps_sl = OPS.tile([128, 512], F32, tag=f"o{sl}")
                        for m in range(HP):
                            nc.tensor.matmul(
                                o_ps_sl[:], vt[:, 2 * m : 2 * m + 2, :],
                                pt[m][:, :, cs],
                                start=(m == 0), stop=(m == HP - 1),
                                perf_mode=DR,
                            )
                        osb = OSB.tile([128, 512], F32, tag="osb")
                        nc.vector.scalar_tensor_tensor(
                            osb[:], o_ps_sl[:], 1.0 / PSC, xb[nt][:, cs],
                            ALU.mult, ALU.add,
                        )
                        nc.sync.dma_start(
                            out_t[nt * 128 : (nt + 1) * 128, cs], osb[:]
                        )

    _split_waits(nc)
    return nc


_NC_CACHE = None
_LAST_IN_MAPS = None


def kernel(
    hidden_states, memory_keys, memory_values, attention_mask, Wq, Wout,
    ln_gamma, ln_beta,
):
    global _NC_CACHE
    if _NC_CACHE is None:
        _NC_CACHE = build_nc()
    nc = _NC_CACHE

    f32 = np.float32
    bf16 = ml_dtypes.bfloat16
    x = np.asarray(hidden_states, dtype=f32).reshape(B * S, HID)
    gamma = np.asarray(ln_gamma, dtype=f32)
    beta = np.asarray(ln_beta, dtype=f32)
    Wq = np.asarray(Wq, dtype=f32)
    Wout = np.asarray(Wout, dtype=f32)

    Wqg = Wq * gamma[None, :]                      # [o, i]
    W2h = Wqg.reshape(HEADS, DH, HID)              # [h, d, i]
    bq = (Wq @ beta).reshape(HEADS, DH)            # [h, d]
    Woh = Wout.reshape(HID, HEADS, DH).transpose(1, 2, 0)  # [h, d, j]

    aqs, vwqs, biases = [], [], []
    for b in range(B):
        Kb = np.asarray(memory_keys[b], dtype=f32)
        Vb = np.asarray(memory_values[b], dtype=f32)
        Kh = Kb.reshape(SLOTS, HEADS, DH).transpose(1, 0, 2)  # [h, t, d]
        Vh = Vb.reshape(SLOTS, HEADS, DH).transpose(1, 0, 2)  # [h, t, d]

        A = np.matmul(Kh, W2h).transpose(0, 2, 1)  # [h, i, t]
        colA = A.sum(axis=1)                       # [h, t]
        aq = np.zeros((HEADS, 128, 2 * KP, 128), dtype=f32)
        aq[:, :, :KT, :] = A.reshape(HEADS, KT, 128, 128).transpose(0, 2, 1, 3)
        aq[:, 0, KT, :] = colA
        aqs.append(aq.astype(E4))

        VWo = np.matmul(Vh, Woh)                   # [h, t, j]
        vwq = VWo.reshape(HEADS, SLOTS, NT, 128).transpose(2, 1, 0, 3)
        vwqs.append(np.ascontiguousarray(vwq).astype(E4))

        m = np.asarray(attention_mask[b]).astype(bool)           # [t]
        bqK = np.einsum("htd,hd->ht", Kh, bq)                    # [h, t]
        bias = SCALE * bqK.T + np.where(m, 0.0, -1e30)[:, None]  # [t, h]
        biases.append(np.ascontiguousarray(bias).astype(f32))

    in_maps = []
    for c in range(NC_):
        rows = slice(c * SC, (c + 1) * SC)
        xt = np.ascontiguousarray(x[rows].T).astype(bf16)  # [HID, SC]
        b = (c * SC) // S
        in_maps.append(
            dict(xtb=xt, aq=aqs[b], vwq=vwqs[b], biasv=biases[b])
        )

    global _LAST_IN_MAPS
    _LAST_IN_MAPS = in_maps
    from concourse import bass2jax

    results = bass2jax.run_bass_via_pjrt(nc, in_maps, n_cores=NC_)

    out = np.empty((B * S, HID), dtype=f32)
    for c in range(NC_):
        out[c * SC : (c + 1) * SC] = results[c]["outt"].T
    return out.reshape(B, S, HID)
